# revision 28
# baseline (speedup 1.0000x reference)
"""BlockSparseLinear forward on 8 Trainium2 NeuronCores (bf16 pipeline).

Computes out = x @ (weight * expand(block_mask))^T + bias for
x [8192, 4096] f32, weight [4096, 4096] f32, bias [4096] f32,
block_mask [128, 128] int32 (32x32 blocks).

Sharding: data-parallel over rows of x -- each of the 8 cores gets a
1024-row slice of x and the full weight / bias / block_mask
(replicated).  No collectives; per-core out^T [4096, 1024] is
transposed and concatenated on the host.

Layout/precision strategy (vs the f32r baseline, 533.9us; this version
measures ~479us, ~444us of which is the PE floor of 2048 matmuls):
  * x and weight ship as bf16 (host-side dtype cast + pure index
    permutations).  bf16 matmuls run at the same 1 cycle/row as f32r,
    but the bf16 stationary enables the PE's Fast Weight Load path
    (f32r counts as FP32_HIGH, which disables FWL): LDWEIGHTS drops
    ~187ns -> ~97ns and hides under the matmul, taking the warm matmul
    cadence from 227ns to the 216ns floor.  DMA traffic halves.
  * Mask expansion is 2 single-DMA partition-broadcasts straight from
    the host-provided maskB layout (maskB[q,t,ob] = maskT[4t+q,ob], so
    every destination partition reads one contiguous run) -- no PE
    selection matmuls, nothing on the vector engine's critical path.
    PSUM is left entirely to the matmul accumulators.
  * 9 dependency-free warmup matmuls on a memset tile run during the
    framework preamble so the HAM clock-gate opens (1.2 -> 2.4 GHz)
    before the first real matmul.
  * Startup: the first 4 output tiles are interleaved across the 8
    x-chunks as they stream in (phase A), so the PE runs at full rate
    while x loads instead of stalling on the full contraction of
    output tile 0.  Remaining 28 output tiles run dense (phase B),
    with weight pieces prefetched two tiles ahead.
  * Per 128-output tile: DVE multiplies the weight tile by the
    partition-replicated mask (bf16, broadcast access pattern); 64
    bf16 matmuls [128x128]x[128x512] accumulate out^T in fp32 PSUM;
    bias is added during PSUM->SBUF eviction on the scalar engine.
    Phase-A evictions instead ride the vector engine so the Tile
    scheduler cannot queue them behind ring-gated DMA issue ops --
    their PSUM banks must free fast for the first phase-B tiles.
  * The last output tile's matmuls are ng-split so its first eviction
    and store overlap the remaining matmuls (shorter tail).

All reference arithmetic -- mask application, matmuls, bias add --
runs on device; host work is dtype casts and index permutations.
"""
import os
import sys

import ml_dtypes
import numpy as np

sys.path.insert(0, "/opt/trn_rl_repo")

from contextlib import ExitStack

import concourse.bass as bass
import concourse.mybir as mybir
import concourse.tile as tile
from concourse import bacc
from concourse.bass_utils import run_bass_kernel_spmd

N_CORES = 8
BS = 32
P = 128

# Filled by kernel() after a profiled run (test harness convenience).
LAST_EXEC_TIME_NS = None
LAST_RESULTS = None

F32 = mybir.dt.float32
BF16 = mybir.dt.bfloat16
I32 = mybir.dt.int32


def _build_program(n_rows, IN, OUT):
    """Per-core SPMD program.  Inputs:
      xq     [NQ, NG, 128, QI, NFREE] bf16  xq[c,ng,p,it,n] = x[ng*NFREE+n, (c*QI+it)*128+p]
      wq     [OT, 128, IT, 128] bf16        wq[ot,p,it,o]   = weight[ot*128+o, it*128+p]
      maskT  [IB, OB] bf16                  block_mask.T
      bias_r [128, OT] f32                  bias_r[p,ot]    = bias[ot*128+p]
    Output outT [OUT, n_rows] f32 (outT[o,n] = out[n,o])."""
    IT = IN // P           # contraction tiles
    OT = OUT // P          # output tiles
    TG = IT // 4           # tile groups (one masked-weight tile per tg)
    NFREE = min(512, n_rows)
    NG = n_rows // NFREE
    IB = IN // BS
    OB = OUT // BS
    QI = 4                 # i-tiles per x chunk
    NQ = IT // QI          # x chunks; phase A round c consumes chunk c
    AOT = 4                # output tiles interleaved in phase A
    PIECE = 4              # i-tiles per weight DMA (= one tile group)
    NPC = IT // PIECE      # weight pieces per output tile
    assert IB <= P and OB <= P and QI * NQ == IT and TG == NQ
    assert PIECE == 4 and NPC == TG

    nc = bacc.Bacc("TRN2", target_bir_lowering=False, debug=False,
                   num_devices=N_CORES)
    xq_d = nc.dram_tensor("xq", [NQ, NG, P, QI, NFREE], BF16,
                          kind="ExternalInput")
    wq_d = nc.dram_tensor("wq", [OT, P, IT, P], BF16, kind="ExternalInput")
    # maskB[q, t, ob] = maskT[4t + q, ob]: partition p = 32q + r of mrep
    # reads the contiguous 8KB row maskB[q] (fast broadcast descriptors).
    mask_d = nc.dram_tensor("maskB", [4, IT, OB], BF16, kind="ExternalInput")
    bias_d = nc.dram_tensor("bias_r", [P, OT], F32, kind="ExternalInput")
    out_d = nc.dram_tensor("outT", [OUT, n_rows], F32, kind="ExternalOutput")

    with tile.TileContext(nc) as tc, ExitStack() as ctx:
        const = ctx.enter_context(tc.tile_pool(name="const", bufs=1))
        xtp = ctx.enter_context(tc.tile_pool(name="xt", bufs=1))
        mrp = ctx.enter_context(tc.tile_pool(name="mrep", bufs=1))
        wnt = ctx.enter_context(tc.tile_pool(name="wnt", bufs=44))
        # 12 bufs: the ng-split passes (phase A round 0, first/last phase-B
        # tiles) keep up to 8 masked-weight tiles live at once, plus
        # run-ahead slack.
        wtm = ctx.enter_context(tc.tile_pool(name="wtm", bufs=12))
        osb = ctx.enter_context(tc.tile_pool(name="osb", bufs=3))
        ppo = ctx.enter_context(tc.tile_pool(name="ppo", bufs=8, space="PSUM"))

        # ---- PE warm-up: dependency-light matmuls during the framework
        # preamble so the HAM clock-gate opens (K=8/8 @ 2.4GHz) before the
        # first real matmul instead of ~3.4us into the main loop.
        dum = const.tile([P, P + NFREE], BF16)
        nc.vector.memset(dum[:], 0.0)
        wup = ppo.tile([P, NFREE], F32, tag="ppo", name="warmup")
        for _ in range(9):
            nc.tensor.matmul(wup[:], dum[:, 0:P], dum[:, P:P + NFREE],
                             start=True, stop=True)

        # ---- mask partition-broadcast, split across both HWDGE rings ----
        # mrep[p, it, ob] = maskT[4*it + p//32, ob]: partition p of i-tile
        # it holds input block ib = 4*it + p//32.  Each destination
        # partition reads one contiguous maskB row (fast descriptors); the
        # two t-halves are separate tiles so the first masked-weight
        # multiply only waits on the first half.
        HT = IT // 2
        mrepA = mrp.tile([P, HT, OB], BF16, name="mrepA")
        mrepB = mrp.tile([P, HT, OB], BF16, name="mrepB")

        def mask_bcast_src(t0, t1):
            # [4, HT, OB] slice -> [4, 32(bcast), HT, OB]: dst partition
            # p = 32q + r reads maskB[q, t0:t1] (contiguous), one DMA total.
            return mask_d[:, t0:t1] \
                .rearrange("q (t x) o -> q x t o", x=1) \
                .broadcast_to([4, 32, t1 - t0, OB])

        nc.scalar.dma_start(mrepA[:], mask_bcast_src(0, HT))

        def load_piece(ot, pc):
            t = wnt.tile([P, PIECE, P], BF16, tag="wnt", name=f"w_{ot}_{pc}")
            nc.scalar.dma_start(t[:], wq_d[ot, :, pc * PIECE:(pc + 1) * PIECE, :])
            return t

        # Phase A weights piece-major so round 0's dependencies land first.
        wpiece = {}
        for pc in range(NPC):
            for ot in range(AOT):
                wpiece[(ot, pc)] = load_piece(ot, pc)
            if pc == 1:
                # second mask half: needed from round TG//2 (~35us in)
                nc.scalar.dma_start(mrepB[:], mask_bcast_src(HT, IT))

        # ---- x stream (sync ring), chunk-major in consumption order ----
        xq = [[xtp.tile([P, QI, NFREE], BF16, name=f"xq_{c}_{ng}",
                        tag=f"xq_{c}_{ng}") for ng in range(NG)]
              for c in range(NQ)]
        for c in range(NQ):
            for ng in range(NG):
                nc.sync.dma_start(xq[c][ng][:], xq_d[c, ng])
        # Bias rides the sync ring after x; first needed at ~65us.
        bias_sb = const.tile([P, OT], F32)
        nc.sync.dma_start(bias_sb[:], bias_d[:])

        def xq_slice(it, ng):
            return xq[it // QI][ng][:, it % QI, :]

        def make_wm(ot, tg):
            wm = wtm.tile([P, 4, P], BF16, tag="wtm")
            wsrc = wpiece[(ot, tg)]
            mr, toff = (mrepA, tg * 4) if tg < TG // 2 else \
                (mrepB, tg * 4 - HT)
            m_ap = mr[:, toff:toff + 4, ot * 4:ot * 4 + 4] \
                .broadcast_to([P, 4, 4, BS])
            nc.vector.tensor_tensor(
                wm[:].rearrange("p a (b c) -> p a b c", c=BS),
                wsrc[:].rearrange("p a (b c) -> p a b c", c=BS),
                m_ap, op=mybir.AluOpType.mult)
            return wm

        def mm_group(po, tg, wm, first, last, ngs=None):
            for j in range(4):
                it = tg * 4 + j
                for ng in (range(NG) if ngs is None else ngs):
                    nc.tensor.matmul(po[ng][:], wm[:, j, :], xq_slice(it, ng),
                                     start=(first and j == 0),
                                     stop=(last and j == 3))

        def evict(po, ot, ngs=None, eng="scalar"):
            # eng="vector": bias-add on DVE.  Used for the phase-A
            # evictions so they cannot be scheduled behind ring-gated DMA
            # issue ops on the scalar queue (PSUM banks must free fast for
            # the first phase-B tiles).
            for ng in (range(NG) if ngs is None else ngs):
                ob_t = osb.tile([P, NFREE], F32, tag="osb")
                if eng == "vector":
                    nc.vector.tensor_tensor(
                        ob_t[:], po[ng][:],
                        bias_sb[:, ot:ot + 1].broadcast_to([P, NFREE]),
                        op=mybir.AluOpType.add)
                else:
                    nc.scalar.activation(ob_t[:], po[ng][:],
                                         mybir.ActivationFunctionType.Identity,
                                         bias=bias_sb[:, ot:ot + 1], scale=1.0)
                nc.sync.dma_start(
                    out_d[ot * P:(ot + 1) * P, ng * NFREE:(ng + 1) * NFREE],
                    ob_t[:])

        # ---- phase A: output tiles 0..AOT-1 interleaved across x chunks ----
        poA = {ot: [ppo.tile([P, NFREE], F32, tag="ppo",
                             name=f"poA_{ot}_{ng}") for ng in range(NG)]
               for ot in range(AOT)}
        for c in range(NQ):
            tg = c  # chunk c holds exactly the i-tiles of tile group c
            if c == 0:
                # ng-split: all ng=0 passes first so the first matmuls only
                # need xq[0][0], which lands ~3us before xq[0][1].
                wms = [make_wm(ot, tg) for ot in range(AOT)]
                for ng in range(NG):
                    for ot in range(AOT):
                        mm_group(poA[ot], tg, wms[ot], first=True, last=False,
                                 ngs=[ng])
                continue
            if c == NQ - 1:
                # Last round: build all masked-weight tiles first so the
                # DVE evictions (emitted per-ot below) cannot delay them;
                # banks then free for phase B during this round.
                wms = [make_wm(ot, tg) for ot in range(AOT)]
                for ot in range(AOT):
                    mm_group(poA[ot], tg, wms[ot], first=False, last=True)
                    evict(poA[ot], ot, eng="vector")
                continue
            for ot in range(AOT):
                wm = make_wm(ot, tg)
                mm_group(poA[ot], tg, wm, first=False, last=False)
            # Prefetch the first phase-B weight tiles mid-phase-A so their
            # issue ops (and any buffer waits) clear the scalar queue before
            # the phase-A evictions enter it.
            if c == 3 or c == 5:
                pot = AOT + (c - 3) // 2
                for pc in range(NPC):
                    wpiece[(pot, pc)] = load_piece(pot, pc)

        # ---- phase B: remaining output tiles, x fully resident ----
        # Weight pieces are prefetched two output tiles ahead.
        for ot in range(AOT, OT):
            if ot + 2 < OT:
                for pc in range(NPC):
                    wpiece[(ot + 2, pc)] = load_piece(ot + 2, pc)
            po = [ppo.tile([P, NFREE], F32, tag="ppo", name=f"po_{ot}_{ng}")
                  for ng in range(NG)]
            if ot == AOT or ot == OT - 1:
                # ng-split passes.  For the first phase-B tile, the ng=0
                # pass starts on the warmup-freed PSUM bank while the
                # phase-A banks drain (kills the A->B gap); for the last
                # tile, ng=0's eviction overlaps ng=1's matmuls and the
                # final ng=1 eviction is chunked, trimming the tail.
                wms = [make_wm(ot, tg) for tg in range(TG)]
                for ng in range(NG):
                    for tg in range(TG):
                        mm_group(po, tg, wms[tg], first=(tg == 0),
                                 last=(tg == TG - 1), ngs=[ng])
                    if ot == OT - 1:
                        if ng == NG - 1:
                            for h in range(2):
                                HF = NFREE // 2
                                ob_t = osb.tile([P, HF], F32, tag="osbh",
                                                name=f"obh_{h}")
                                nc.scalar.activation(
                                    ob_t[:], po[ng][:, h * HF:(h + 1) * HF],
                                    mybir.ActivationFunctionType.Identity,
                                    bias=bias_sb[:, ot:ot + 1], scale=1.0)
                                nc.sync.dma_start(
                                    out_d[ot * P:(ot + 1) * P,
                                          ng * NFREE + h * HF:
                                          ng * NFREE + (h + 1) * HF],
                                    ob_t[:])
                        else:
                            evict(po, ot, ngs=[ng])
                if ot == AOT:
                    evict(po, ot)
            else:
                for tg in range(TG):
                    wm = make_wm(ot, tg)
                    mm_group(po, tg, wm, first=(tg == 0), last=(tg == TG - 1))
                evict(po, ot)

    nc.finalize()
    return nc


def _tile_x(x_slice_bf, IN, n_rows):
    """xq[c, ng, p, it, n] = x[ng*NFREE+n, (c*QI+it)*128+p] (bf16 in/out)."""
    QI = 4
    NQ = (IN // P) // QI
    NFREE = min(512, n_rows)
    NG = n_rows // NFREE
    xt = x_slice_bf.T                                  # [IN, n_rows]
    xq = xt.reshape(NQ, QI, P, NG, NFREE).transpose(0, 3, 2, 1, 4)
    return np.ascontiguousarray(xq)


def _install_profile_hook():
    """Provide antenv.axon_hooks + the ctypes NTFF hook (profiling only)."""
    import types

    try:
        from antenv import axon_hooks  # noqa: F401
    except ImportError:
        import antenv

        mod = types.ModuleType("antenv.axon_hooks")
        _h = [None]
        mod.set_axon_ntff_profile_hook = lambda h: _h.__setitem__(0, h)
        mod.get_axon_ntff_profile_hook = lambda: _h[0]
        sys.modules["antenv.axon_hooks"] = mod
        antenv.axon_hooks = mod
    from antenv.axon_hooks import (
        get_axon_ntff_profile_hook,
        set_axon_ntff_profile_hook,
    )

    if get_axon_ntff_profile_hook() is None:
        so_path = "/opt/axon/libaxon_pjrt.so"
        if os.path.exists(so_path):
            from trn_agent_boot.trn_boot import _ntff_profile_via_ctypes

            set_axon_ntff_profile_hook(_ntff_profile_via_ctypes(so_path))

    # Zero-egress container: artifact upload would fail; keep it local.
    import concourse.bass_utils as bu

    bu.upload_artifacts = lambda tmpdir: tmpdir


def kernel(x, weight, bias, block_mask):
    global LAST_EXEC_TIME_NS, LAST_RESULTS
    x = np.ascontiguousarray(np.asarray(x, dtype=np.float32))
    weight = np.ascontiguousarray(np.asarray(weight, dtype=np.float32))
    bias = np.asarray(bias, dtype=np.float32)
    block_mask = np.ascontiguousarray(np.asarray(block_mask, dtype=np.int32))

    N, IN = x.shape
    OUT = weight.shape[0]
    assert N % N_CORES == 0
    n_rows = N // N_CORES
    IT, OT = IN // P, OUT // P

    bf16 = ml_dtypes.bfloat16
    xb = x.astype(bf16)
    wb = weight.astype(bf16)
    # wq[ot, p, it, o] = weight[ot*128+o, it*128+p]
    wq = np.ascontiguousarray(wb.reshape(OT, P, IT, P).transpose(0, 3, 2, 1))
    # maskB[q, t, ob] = block_mask[ob, 4t + q] (i.e. maskT[4t+q, ob])
    maskT = block_mask.T.astype(bf16)
    maskB = np.ascontiguousarray(
        maskT.reshape(IT, 4, OUT // BS).transpose(1, 0, 2))
    bias_r = np.ascontiguousarray(bias.reshape(OT, P).T)

    nc = _build_program(n_rows, IN, OUT)
    in_maps = [{
        "xq": _tile_x(xb[c * n_rows:(c + 1) * n_rows, :], IN, n_rows),
        "wq": wq,
        "maskB": maskB,
        "bias_r": bias_r,
    } for c in range(N_CORES)]

    trace = bool(int(os.environ.get("BASS_KERNEL_TRACE", "0")))
    if trace:
        _install_profile_hook()
    res = run_bass_kernel_spmd(nc, in_maps, list(range(N_CORES)), trace=trace)
    LAST_EXEC_TIME_NS = res.exec_time_ns
    LAST_RESULTS = res

    out = np.empty((N, OUT), dtype=np.float32)
    for c in range(N_CORES):
        out[c * n_rows:(c + 1) * n_rows, :] = res.results[c]["outT"].T
    return out
